# revision 8
# baseline (speedup 1.0000x reference)
"""Trainium2 Bass kernel for BoostedPointPairNet2.

Model (per (b, d) group, m = 128 points, din = 3):
  H1(i,j) = relu(W1A @ x_j + W1B @ x_i + b1)          (64)
  H2(i,j) = relu(W2 @ H1 + b2)                        (128)
  G(i,j)  = W3 @ H2                                    (256, b3 deferred)
  P       = max_{i,j} G + b3                           (256)
  Y       = V3 @ relu(V2 @ relu(V1 @ P + c1) + c2) + c3  (40)
  out[b]  = max_d Y[b, d]

Sharding: 16 (b, d) groups over 8 cores, 2 groups per core. Weights
replicated. Each core returns its two groups' Y rows; the host does the
final max over d.

v2 dataflow (lockstep groups, all-direct drains):
  The two groups run LOCKSTEP, alternating iterations (even iter -> group
  0, odd -> group 1), 4 j-pairs per iteration. PSUM is ONE [128, 4096]
  fp32 mega-tile = 4 universal [1024]-col slots (A B C D) on a period-2
  rotation:
    even pair of iters: l2 -> A, B;  G(iter even) -> C+D,  G(odd) -> A+B
    odd  pair of iters: l2 -> C, D;  G -> A+B then C+D
  so the h2 relu runs as ONE [2048] ACT op per iter-pair and every G
  drain is ONE contiguous [2048] DVE reduce_max straight from PSUM into
  per-group racc columns (fp32, no fp16 copy path at all).

  Engine balance (HW-measured per-op): DVE carries the 32 drains
  (~2.28us each) plus a small share of H1; ACT carries the 16 relu pairs
  (~1.97us) plus ~94% of the 128 H1 builds (activation with per-partition
  uu bias, ~0.40us each). PE (~53% busy) gets a warmup burst at t=0 so
  the HAM clock-gate reaches 2.4 GHz before the pipeline starts.

  The F-MLP tail is batched across both groups (N=2 matmuls) and runs
  once at the end.
"""

import numpy as np
import ml_dtypes

import bass_rust
import concourse.bass as bass
import concourse.mybir as mybir
from concourse.tile import TileContext
from concourse.bass_utils import run_bass_kernel_spmd

BF16 = ml_dtypes.bfloat16
F32 = np.float32
DT = mybir.dt
ALU = mybir.AluOpType
AX = mybir.AxisListType
RELU = mybir.ActivationFunctionType.Relu

N_CORES = 8
B, N, DIN = 4, 512, 3
D = 4                    # boost factor
M = N // D               # 128 points per group
GROUPS_PER_CORE = 2
JP = M // 2              # 64 stacked j-pairs per group
NITER = 32               # lockstep iterations (16 per group, 4 jp each)
NWARM = 26               # warmup junk matmuls (N=128, ~107ns cold each)
# jp-index stride for H1 ops done by the DVE (rest go to ACT)
H1_DVE_MOD = 16


def _split_multi_waits(nc):
    """This walrus build accepts at most ONE sync wait per instruction;
    hoist extra waits onto same-engine nops inserted before the offender."""
    seq = 0
    for fn in nc.m.functions:
        for bb in fn.blocks:
            new = []
            changed = False
            for ins in bb.instructions:
                si = ins.sync_info
                waits = list(si.on_wait) if si is not None and si.on_wait else []
                if len(waits) > 1:
                    changed = True
                    for w in waits[:-1]:
                        seq += 1
                        new.append(
                            mybir.InstNoOp(
                                name=f"I-wsplit-{seq}",
                                engine=ins.engine,
                                sync_info=bass_rust.SyncInfo(
                                    on_wait=[w], on_update=[]
                                ),
                            )
                        )
                    ins.sync_info = bass_rust.SyncInfo(
                        on_wait=[waits[-1]], on_update=list(si.on_update or [])
                    )
                new.append(ins)
            if changed:
                bb.instructions = new


# ---------------------------------------------------------------------------
# Device program
# ---------------------------------------------------------------------------
def _build_program():
    nc = bass.Bass(
        "TRN2", target_bir_lowering=False, debug=False, num_devices=N_CORES
    )

    xt = nc.declare_dram_parameter(
        "xt", [GROUPS_PER_CORE, DIN, M], DT.bfloat16, isOutput=False
    )
    # col 0:128 w2a=[W2T;0], 128:256 w2b=[0;W2T], 256:384 w3a, 384:512 w3b
    wblob = nc.declare_dram_parameter("wblob", [128, 512], DT.bfloat16, isOutput=False)
    # col 0:128 w1a_even=[W1AT|0], 128:256 w1a_odd=[0|W1AT], 256:384 w1b2=[W1BT|W1BT]
    w1blob = nc.declare_dram_parameter("w1blob", [DIN, 384], DT.bfloat16, isOutput=False)
    # v1t (2x512) | v2t (4x256) | v3t (2x40)
    vblob = nc.declare_dram_parameter("vblob", [128, 2128], DT.float16, isOutput=False)
    # col 0 b1st, 1 b2c, 2:4 b3_2, 4:12 c1_42 (4mm x 2g), 12:16 c2_22, 16 c3
    cblob = nc.declare_dram_parameter("cblob", [128, 17], DT.float32, isOutput=False)
    y_out = nc.declare_dram_parameter(
        "y", [GROUPS_PER_CORE, 40], DT.float32, isOutput=True
    )

    with TileContext(nc) as tc:
        with (
            tc.tile_pool(name="singles", bufs=1) as singles,
            tc.tile_pool(name="xtp", bufs=2) as xtp,
            tc.tile_pool(name="vup", bufs=4) as vup,
            tc.tile_pool(name="h1p", bufs=4) as h1pool,
            tc.tile_pool(name="h2p", bufs=3) as h2pool,
            tc.tile_pool(name="fmlp", bufs=10) as fmlp,
            tc.tile_pool(name="psum", bufs=1, space="PSUM") as psum,
        ):
            # The whole PSUM as one tile; 4 slots of 1024 fp32 cols.
            mega = psum.tile([128, 4096], DT.float32, tag="mega")

            def slot(k):
                return mega[:, 1024 * k : 1024 * (k + 1)]

            # ---- input DMAs (sync/gpsimd/tensor queues only) ----
            sb_xts = []
            for g in range(GROUPS_PER_CORE):
                t = xtp.tile([DIN, M], DT.bfloat16)
                sb_xts.append(t)
            nc.sync.dma_start(out=sb_xts[0], in_=xt[0])
            sb_w1 = singles.tile([DIN, 384], DT.bfloat16, tag="w1blob")
            nc.gpsimd.dma_start(out=sb_w1, in_=w1blob[:, :])
            sb_c = singles.tile([128, 17], DT.float32, tag="cblob")
            nc.sync.dma_start(out=sb_c, in_=cblob[:, :])
            sb_w = singles.tile([128, 512], DT.bfloat16, tag="wblob")
            nc.gpsimd.dma_start(out=sb_w, in_=wblob[:, :])
            nc.sync.dma_start(out=sb_xts[1], in_=xt[1])
            sb_v = singles.tile([128, 2128], DT.float16, tag="vblob")
            nc.gpsimd.dma_start(out=sb_v, in_=vblob[:, :])

            # ---- PE warmup burst (junk matmuls while DMAs land) ----
            wjunk = singles.tile([128, 128], DT.bfloat16, tag="wjunk")
            nc.gpsimd.memset(wjunk, 0.0)
            for _ in range(NWARM):
                nc.tensor.matmul(
                    mega[:, 3072:3200], lhsT=wjunk, rhs=wjunk,
                    start=True, stop=True,
                )

            # dummy relu to hoist ACT_TABLE_LOAD into the init shadow
            warm = singles.tile([1, 1], DT.float32, tag="warm")
            nc.gpsimd.memset(warm, 0.0)
            nc.scalar.activation(out=warm, in_=warm, func=RELU)

            sb_w2a, sb_w2b = sb_w[:, 0:128], sb_w[:, 128:256]
            sb_w3a, sb_w3b = sb_w[:, 256:384], sb_w[:, 384:512]
            sb_w1a_e, sb_w1a_o = sb_w1[:, 0:128], sb_w1[:, 128:256]
            sb_w1b2 = sb_w1[:, 256:384]
            sb_b1st = sb_c[:, 0:1]
            sb_b2c = sb_c[:, 1:2]
            sb_b3_2 = sb_c[:, 2:4]
            sb_c1_42 = sb_c[:, 4:12]
            sb_c2_22 = sb_c[:, 12:16]
            sb_c3c = sb_c[0:40, 16:17]

            def v1t(k):  # [128, 512] fp16, k in 0..1
                return sb_v[:, 512 * k : 512 * (k + 1)]

            def v2t(k):  # [128, 256] fp16, k in 0..3
                return sb_v[:, 1024 + 256 * k : 1024 + 256 * (k + 1)]

            def v3t(k):  # [128, 40] fp16, k in 0..1
                return sb_v[:, 2048 + 40 * k : 2048 + 40 * (k + 1)]

            # ---- both groups' prep: stacked v (V2) and u+b1 (UU) ----
            # prep psum carved from slots C, D (first pipeline use is late)
            sb_v2s, sb_uus = [], []
            for g in range(GROUPS_PER_CORE):
                sb_xt = sb_xts[g]
                xt_eo = sb_xt.rearrange("k (j two) -> k two j", two=2)
                v2ps = mega[:, 2048 + 512 * g : 2048 + 512 * g + 128]
                nc.tensor.matmul(v2ps, lhsT=sb_w1b2, rhs=sb_xt, start=True, stop=True)
                uups = mega[:, 3200 + 256 * g : 3200 + 256 * g + 64]
                nc.tensor.matmul(
                    uups, lhsT=sb_w1a_e, rhs=xt_eo[:, 0, :], start=True, stop=False
                )
                nc.tensor.matmul(
                    uups, lhsT=sb_w1a_o, rhs=xt_eo[:, 1, :], start=False, stop=True
                )
                sb_v2 = vup.tile([128, M], DT.bfloat16, tag=f"v2_{g}")
                nc.vector.tensor_copy(out=sb_v2, in_=v2ps)
                sb_uu = vup.tile([128, JP], DT.float32, tag=f"uu_{g}")
                nc.vector.tensor_scalar_add(out=sb_uu, in0=uups, scalar1=sb_b1st)
                sb_v2s.append(sb_v2)
                sb_uus.append(sb_uu)

            # per-group drain accumulators: [128, 4(a,b,a,b), 16 drains]
            raccs = []
            for g in range(GROUPS_PER_CORE):
                racc = vup.tile([128, 4, NITER // 2], DT.float32, tag=f"racc{g}")
                raccs.append(racc)

            # ---- main lockstep pipeline ----
            # slot schedule, period 2 iteration-pairs:
            #  pair k even: l2 -> A, B ; G(2k) -> C+D ; G(2k+1) -> A+B
            #  pair k odd : l2 -> C, D ; G(2k) -> A+B ; G(2k+1) -> C+D
            for k in range(NITER // 2):
                i0, i1 = 2 * k, 2 * k + 1
                it = k  # per-group iteration index (4 jp)
                if k % 2 == 0:
                    l2s0, l2s1, ga0, ga1 = 0, 1, 2, 0
                else:
                    l2s0, l2s1, ga0, ga1 = 2, 3, 0, 2
                # H1 for both iterations of the pair
                h1s = []
                for gi, (g, l2slot) in enumerate(((0, l2s0), (1, l2s1))):
                    sb_v2, sb_uu = sb_v2s[g], sb_uus[g]
                    h1 = h1pool.tile([128, 512], DT.bfloat16)
                    for jj in range(4):
                        jp = it * 4 + jj
                        dst = h1[:, jj * M : (jj + 1) * M]
                        if (jp * 2 + g) % H1_DVE_MOD == 0:
                            nc.vector.tensor_scalar(
                                out=dst, in0=sb_v2,
                                scalar1=sb_uu[:, jp : jp + 1], scalar2=0.0,
                                op0=ALU.add, op1=ALU.max,
                            )
                        else:
                            nc.scalar.activation(
                                out=dst, in_=sb_v2, func=RELU,
                                bias=sb_uu[:, jp : jp + 1], scale=1.0,
                            )
                    h1s.append(h1)
                    # L2: 4 matmuls N=256 into this iter's l2 slot
                    l2ps = slot(l2slot)
                    nc.tensor.matmul(
                        l2ps[:, 0:256], lhsT=sb_w2a, rhs=h1[:, 0:256],
                        start=True, stop=True,
                    )
                    nc.tensor.matmul(
                        l2ps[:, 512:768], lhsT=sb_w2a, rhs=h1[:, 256:512],
                        start=True, stop=True,
                    )
                    nc.tensor.matmul(
                        l2ps[:, 256:512], lhsT=sb_w2b, rhs=h1[:, 0:256],
                        start=True, stop=True,
                    )
                    nc.tensor.matmul(
                        l2ps[:, 768:1024], lhsT=sb_w2b, rhs=h1[:, 256:512],
                        start=True, stop=True,
                    )

                # fused relu over both iterations' l2 slots ([2048], one op)
                h2 = h2pool.tile([128, 2048], DT.bfloat16)
                nc.scalar.activation(
                    out=h2,
                    in_=mega[:, 1024 * l2s0 : 1024 * l2s0 + 2048],
                    func=RELU, bias=sb_b2c, scale=1.0,
                )

                # L3 + drain for each iteration of the pair
                for gi, (g, gslot) in enumerate(((0, ga0), (1, ga1))):
                    h2a = h2[:, 1024 * gi : 1024 * gi + 512]
                    h2b = h2[:, 1024 * gi + 512 : 1024 * gi + 1024]
                    gpa = slot(gslot)
                    gpb = slot(gslot + 1)
                    nc.tensor.matmul(
                        gpa[:, 0:512], lhsT=sb_w3a, rhs=h2a, start=True, stop=True,
                    )
                    nc.tensor.matmul(
                        gpa[:, 512:1024], lhsT=sb_w3b, rhs=h2a, start=True, stop=True,
                    )
                    nc.tensor.matmul(
                        gpb[:, 0:512], lhsT=sb_w3a, rhs=h2b, start=True, stop=True,
                    )
                    nc.tensor.matmul(
                        gpb[:, 512:1024], lhsT=sb_w3b, rhs=h2b, start=True, stop=True,
                    )
                    # direct drain: [2048] contiguous reduce into racc cols
                    gsl = mega[:, 1024 * gslot : 1024 * gslot + 2048]
                    nc.vector.reduce_max(
                        out=raccs[g][:, :, it : it + 1],
                        in_=gsl.rearrange("p (a b) -> p a b", a=4),
                        axis=AX.X,
                    )

            # ---- P per group, batched F-MLP (N=2) ----
            # racc cols layout (a,b,a,b) -> halves; pb is (half, group)
            pb = fmlp.tile([128, 2, 2], DT.float16, tag="pb")
            for g in range(GROUPS_PER_CORE):
                pm = fmlp.tile([128, 2], DT.float32, tag=f"pm{g}")
                nc.vector.reduce_max(
                    out=pm,
                    in_=raccs[g].rearrange("p (x h) t -> p h x t", h=2),
                    axis=AX.XY,
                )
                nc.vector.tensor_tensor(
                    out=pb[:, :, g], in0=pm, in1=sb_b3_2, op=ALU.add
                )

            y1ps = mega[:, 0:8].rearrange("p (m g) -> p m g", m=4)
            for mm in range(4):
                for kk in range(2):
                    nc.tensor.matmul(
                        y1ps[:, mm, :],
                        lhsT=v1t(kk)[:, mm * 128 : (mm + 1) * 128],
                        rhs=pb[:, kk, :],
                        start=(kk == 0),
                        stop=(kk == 1),
                    )
            y1pre = fmlp.tile([128, 8], DT.float32, tag="y1pre")
            nc.vector.tensor_tensor(
                out=y1pre, in0=mega[:, 0:8], in1=sb_c1_42, op=ALU.add
            )
            y1 = fmlp.tile([128, 4, 2], DT.float16, tag="y1")
            nc.vector.tensor_scalar_max(
                out=y1.rearrange("p m g -> p (m g)"), in0=y1pre, scalar1=0.0
            )

            y2ps = mega[:, 1024:1028].rearrange("p (m g) -> p m g", m=2)
            for mm in range(2):
                for kk in range(4):
                    nc.tensor.matmul(
                        y2ps[:, mm, :],
                        lhsT=v2t(kk)[:, mm * 128 : (mm + 1) * 128],
                        rhs=y1[:, kk, :],
                        start=(kk == 0),
                        stop=(kk == 3),
                    )
            y2pre = fmlp.tile([128, 4], DT.float32, tag="y2pre")
            nc.vector.tensor_tensor(
                out=y2pre, in0=mega[:, 1024:1028], in1=sb_c2_22, op=ALU.add
            )
            y2 = fmlp.tile([128, 2, 2], DT.float16, tag="y2")
            nc.vector.tensor_scalar_max(
                out=y2.rearrange("p m g -> p (m g)"), in0=y2pre, scalar1=0.0
            )

            y3ps = mega[0:40, 2048:2050]
            for kk in range(2):
                nc.tensor.matmul(
                    y3ps,
                    lhsT=v3t(kk)[:, 0:40],
                    rhs=y2[:, kk, :],
                    start=(kk == 0),
                    stop=(kk == 1),
                )
            y3 = fmlp.tile([40, 2], DT.float32, tag="y3")
            nc.vector.tensor_scalar_add(out=y3, in0=y3ps, scalar1=sb_c3c)
            for g in range(GROUPS_PER_CORE):
                nc.sync.dma_start(out=y_out[g, :], in_=y3[:, g])

    _split_multi_waits(nc)
    return nc


# ---------------------------------------------------------------------------
# Host side
# ---------------------------------------------------------------------------
_NC_CACHE = None


def _get_program():
    global _NC_CACHE
    if _NC_CACHE is None:
        _NC_CACHE = _build_program()
    return _NC_CACHE


def _make_in_maps(inputs):
    X = np.asarray(inputs["X"], F32)
    W1 = np.asarray(inputs["W1"], F32)
    b1 = np.asarray(inputs["b1"], F32)
    W2 = np.asarray(inputs["W2"], F32)
    b2 = np.asarray(inputs["b2"], F32)
    W3 = np.asarray(inputs["W3"], F32)
    b3 = np.asarray(inputs["b3"], F32)
    V1 = np.asarray(inputs["V1"], F32)
    c1 = np.asarray(inputs["c1"], F32)
    V2 = np.asarray(inputs["V2"], F32)
    c2 = np.asarray(inputs["c2"], F32)
    V3 = np.asarray(inputs["V3"], F32)
    c3 = np.asarray(inputs["c3"], F32)

    W1A, W1B = W1[:, :DIN], W1[:, DIN:]
    z = np.zeros((DIN, 64), F32)
    w1blob = np.concatenate(
        [W1A.T, z, z, W1A.T, W1B.T, W1B.T], axis=1
    ).astype(BF16)
    z64 = np.zeros((64, 128), F32)
    wblob = np.concatenate(
        [
            np.concatenate([W2.T, z64], axis=0),
            np.concatenate([z64, W2.T], axis=0),
            W3.T[:, 0:128],
            W3.T[:, 128:256],
        ],
        axis=1,
    ).astype(BF16)
    v1t_cols = V1.T.reshape(2, 128, 512).transpose(1, 0, 2).reshape(128, 1024)
    vblob = np.concatenate(
        [v1t_cols,
         V2.T.reshape(4, 128, 256).transpose(1, 0, 2).reshape(128, 1024),
         V3.T.reshape(2, 128, 40).transpose(1, 0, 2).reshape(128, 80)],
        axis=1,
    ).astype(np.float16)
    cblob = np.zeros((128, 17), F32)
    cblob[:, 0] = np.concatenate([b1, b1])
    cblob[:, 1] = b2
    cblob[:, 2:4] = b3.reshape(2, 128).T
    # c1_42: [4 mm, 2 g] duplicated groups
    c1_4 = c1.reshape(4, 128).T           # [128, 4]
    cblob[:, 4:12] = np.repeat(c1_4, 2, axis=1)
    c2_2 = c2.reshape(2, 128).T           # [128, 2]
    cblob[:, 12:16] = np.repeat(c2_2, 2, axis=1)
    cblob[0:40, 16] = c3

    shared = dict(wblob=wblob, w1blob=w1blob, vblob=vblob, cblob=cblob)

    Xv = X.reshape(B, D, M, DIN)
    in_maps = []
    for c in range(N_CORES):
        xts = np.empty((GROUPS_PER_CORE, DIN, M), F32)
        for gi in range(GROUPS_PER_CORE):
            g = 2 * c + gi
            bb, dd = g // D, g % D
            xts[gi] = Xv[bb, dd].T
        in_maps.append(dict(shared, xt=xts.astype(BF16)))
    return in_maps


def _run(inputs, trace=False):
    nc = _get_program()
    in_maps = _make_in_maps(inputs)
    res = run_bass_kernel_spmd(nc, in_maps, list(range(N_CORES)), trace=trace)
    ys = np.stack([res.results[c]["y"] for c in range(N_CORES)])  # [8, 2, 40]
    y16 = ys.reshape(B, D, 40)
    out = y16.max(axis=1).astype(F32)
    return out, res


def kernel(**inputs):
    out, _ = _run(inputs, trace=False)
    return out


# revision 10
# speedup vs baseline: 1.1312x; 1.1312x over previous
"""Trainium2 Bass kernel for BoostedPointPairNet2.

Model (per (b, d) group, m = 128 points, din = 3):
  H1(i,j) = relu(W1A @ x_j + W1B @ x_i + b1)          (64)
  H2(i,j) = relu(W2 @ H1 + b2)                        (128)
  G(i,j)  = W3 @ H2                                    (256, b3 deferred)
  P       = max_{i,j} G + b3                           (256)
  Y       = V3 @ relu(V2 @ relu(V1 @ P + c1) + c2) + c3  (40)
  out[b]  = max_d Y[b, d]

Sharding: 16 (b, d) groups over 8 cores, 2 groups per core; host does the
final max over d.

v3 dataflow — H1 computed ON THE PE via selection-matrix matmuls:
  pre_H1[(e,ch), (q,i)] = sum_jp uT[jp, (e,ch)] * sel[jp, (q,i)]   (K=64)
                        + sum_i' v2T[i', (e,ch)] * iden[i', (q,i)] (K=128)
  where sel is the 0/1 matrix delta(jp == 4*it+q) and iden is I_128
  tiled 4x along columns; b1 is folded into uT via an extra ones row in
  the x input.  One [512]-col relu (ACT) then yields the bf16 stacked h1
  tile, replacing 128 per-j-pair elementwise ops (~48us of engine time)
  with ~14us of otherwise-idle PE time.

  The two groups run lockstep (even iter -> group 0, odd -> group 1),
  4 j-pairs per iteration.  PSUM [128, 4096] fp32 mega-tile layout:
    cols    0:1024  two [512] pre_H1 buffers (banks 0-1)
    cols 1024:2048  single l2 buffer (banks 2-3)
    cols 2048:4096  two [1024] G slots (banks 4-7)
  G drains are per-slot [1024] ops so the DVE stays throughput-bound:
  most tiles are direct DVE reduce_max into per-group racc columns; every
  4th per-group tile takes the ACT-copy (fp16) + DVE tensor_tensor-max
  path to balance the engines (~75us each).  The F-MLP tail is batched
  across both groups (N=2 matmuls) and runs once at the end.
"""

import numpy as np
import ml_dtypes

import bass_rust
import concourse.bass as bass
import concourse.mybir as mybir
from concourse.tile import TileContext
from concourse.bass_utils import run_bass_kernel_spmd

BF16 = ml_dtypes.bfloat16
F32 = np.float32
DT = mybir.dt
ALU = mybir.AluOpType
AX = mybir.AxisListType
RELU = mybir.ActivationFunctionType.Relu

N_CORES = 8
B, N, DIN = 4, 512, 3
D = 4                    # boost factor
M = N // D               # 128 points per group
GROUPS_PER_CORE = 2
JP = M // 2              # 64 stacked j-pairs per group
NITER = 32               # lockstep iterations (16 per group, 4 jp each)
NWARM = 26               # warmup junk matmuls to lift the HAM clock gate
COPY_MOD = 4             # per-group drain k: ACT-copy path if k % COPY_MOD == 3
N_COPY_PG = sum(1 for k in range(NITER) if k % COPY_MOD == 3)
N_DIR_PG = NITER - N_COPY_PG


def _split_multi_waits(nc):
    """This walrus build accepts at most ONE sync wait per instruction;
    hoist extra waits onto same-engine nops inserted before the offender."""
    seq = 0
    for fn in nc.m.functions:
        for bb in fn.blocks:
            new = []
            changed = False
            for ins in bb.instructions:
                si = ins.sync_info
                waits = list(si.on_wait) if si is not None and si.on_wait else []
                if len(waits) > 1:
                    changed = True
                    for w in waits[:-1]:
                        seq += 1
                        new.append(
                            mybir.InstNoOp(
                                name=f"I-wsplit-{seq}",
                                engine=ins.engine,
                                sync_info=bass_rust.SyncInfo(
                                    on_wait=[w], on_update=[]
                                ),
                            )
                        )
                    ins.sync_info = bass_rust.SyncInfo(
                        on_wait=[waits[-1]], on_update=list(si.on_update or [])
                    )
                new.append(ins)
            if changed:
                bb.instructions = new


# ---------------------------------------------------------------------------
# Device program
# ---------------------------------------------------------------------------
def _build_program():
    nc = bass.Bass(
        "TRN2", target_bir_lowering=False, debug=False, num_devices=N_CORES
    )

    # x with an appended ones row (folds b1 into the uT prep matmul)
    xt = nc.declare_dram_parameter(
        "xt", [GROUPS_PER_CORE, DIN + 1, M], DT.bfloat16, isOutput=False
    )
    # col 0:128 w2a=[W2T;0], 128:256 w2b=[0;W2T], 256:384 w3a, 384:512 w3b
    wblob = nc.declare_dram_parameter("wblob", [128, 512], DT.bfloat16, isOutput=False)
    # cols 0:64 u-rhs ([W1A^T; b1]), cols 64:192 w1b2 ([W1B^T|W1B^T]; 0)
    w1blob = nc.declare_dram_parameter("w1blob", [DIN + 1, 192], DT.bfloat16, isOutput=False)
    # sel[jp, it*512 + q*128 + i] = (jp == 4*it + q)
    selb = nc.declare_dram_parameter("selb", [JP, (NITER // 2) * 512], DT.bfloat16, isOutput=False)
    # iden[i', q*128 + i] = (i' == i)
    idenb = nc.declare_dram_parameter("idenb", [M, 4 * M], DT.bfloat16, isOutput=False)
    # v1t (2x512) | v2t (4x256) | v3t (2x40)
    vblob = nc.declare_dram_parameter("vblob", [128, 2128], DT.float16, isOutput=False)
    # col 1 b2c, 2:4 b3_2, 4:12 c1_42 (4mm x 2g), 12:16 c2_22, 16 c3
    cblob = nc.declare_dram_parameter("cblob", [128, 17], DT.float32, isOutput=False)
    y_out = nc.declare_dram_parameter(
        "y", [GROUPS_PER_CORE, 40], DT.float32, isOutput=True
    )

    with TileContext(nc) as tc:
        with (
            tc.tile_pool(name="singles", bufs=1) as singles,
            tc.tile_pool(name="xtp", bufs=2) as xtp,
            tc.tile_pool(name="vup", bufs=8) as vup,
            tc.tile_pool(name="h1p", bufs=3) as h1pool,
            tc.tile_pool(name="h2p", bufs=3) as h2pool,
            tc.tile_pool(name="gcp", bufs=3) as gcpool,
            tc.tile_pool(name="fmlp", bufs=12) as fmlp,
            tc.tile_pool(name="psum", bufs=1, space="PSUM") as psum,
        ):
            # The whole PSUM as one tile.
            mega = psum.tile([128, 4096], DT.float32, tag="mega")

            # ---- input DMAs (sync + gpsimd queues; ACT/DVE stay free) ----
            sb_sel = singles.tile([JP, (NITER // 2) * 512], DT.bfloat16, tag="selb")
            nc.gpsimd.dma_start(out=sb_sel, in_=selb[:, :])
            sb_xts = []
            for g in range(GROUPS_PER_CORE):
                t = xtp.tile([DIN + 1, M], DT.bfloat16)
                sb_xts.append(t)
            nc.sync.dma_start(out=sb_xts[0], in_=xt[0])
            sb_w1 = singles.tile([DIN + 1, 192], DT.bfloat16, tag="w1blob")
            nc.sync.dma_start(out=sb_w1, in_=w1blob[:, :])
            sb_iden = singles.tile([M, 4 * M], DT.bfloat16, tag="idenb")
            nc.sync.dma_start(out=sb_iden, in_=idenb[:, :])
            sb_c = singles.tile([128, 17], DT.float32, tag="cblob")
            nc.sync.dma_start(out=sb_c, in_=cblob[:, :])
            sb_w = singles.tile([128, 512], DT.bfloat16, tag="wblob")
            nc.gpsimd.dma_start(out=sb_w, in_=wblob[:, :])
            nc.sync.dma_start(out=sb_xts[1], in_=xt[1])
            sb_v = singles.tile([128, 2128], DT.float16, tag="vblob")
            nc.gpsimd.dma_start(out=sb_v, in_=vblob[:, :])

            # ---- PE warmup burst (junk matmuls while DMAs land) ----
            wjunk = singles.tile([128, 128], DT.bfloat16, tag="wjunk")
            nc.gpsimd.memset(wjunk, 0.0)
            for _ in range(NWARM):
                nc.tensor.matmul(
                    mega[:, 3200:3328], lhsT=wjunk, rhs=wjunk,
                    start=True, stop=True,
                )

            # dummy relu to hoist ACT_TABLE_LOAD into the init shadow
            warm = singles.tile([1, 1], DT.float32, tag="warm")
            nc.gpsimd.memset(warm, 0.0)
            nc.scalar.activation(out=warm, in_=warm, func=RELU)

            sb_w2a, sb_w2b = sb_w[:, 0:128], sb_w[:, 128:256]
            sb_w3a, sb_w3b = sb_w[:, 256:384], sb_w[:, 384:512]
            sb_urhs = sb_w1[:, 0:64]
            sb_w1b2 = sb_w1[0:DIN, 64:192]
            sb_b2c = sb_c[:, 1:2]
            sb_b3_2 = sb_c[:, 2:4]
            sb_c1_42 = sb_c[:, 4:12]
            sb_c2_22 = sb_c[:, 12:16]
            sb_c3c = sb_c[0:40, 16:17]

            def v1t(k):  # [128, 512] fp16, k in 0..1
                return sb_v[:, 512 * k : 512 * (k + 1)]

            def v2t(k):  # [128, 256] fp16, k in 0..3
                return sb_v[:, 1024 + 256 * k : 1024 + 256 * (k + 1)]

            def v3t(k):  # [128, 40] fp16, k in 0..1
                return sb_v[:, 2048 + 40 * k : 2048 + 40 * (k + 1)]

            # ---- per-group prep: uT [64,128] and v2T [128,128] in SBUF bf16
            # (psum carved from the G regions, consumed before first L3) ----
            uT_sbs, v2T_sbs = [], []
            for g in range(GROUPS_PER_CORE):
                sb_xt = sb_xts[g]
                xt_eo = sb_xt.rearrange("k (j two) -> k two j", two=2)
                uTps = mega[0:JP, 2048 + 512 * g : 2048 + 512 * g + 128]
                nc.tensor.matmul(
                    uTps[:, 0:64], lhsT=xt_eo[:, 0, :], rhs=sb_urhs,
                    start=True, stop=True,
                )
                nc.tensor.matmul(
                    uTps[:, 64:128], lhsT=xt_eo[:, 1, :], rhs=sb_urhs,
                    start=True, stop=True,
                )
                uT_sb = vup.tile([JP, 128], DT.bfloat16, tag=f"uT{g}")
                nc.vector.tensor_copy(out=uT_sb, in_=uTps)
                v2Tps = mega[:, 3072 + 512 * g : 3072 + 512 * g + 128]
                nc.tensor.matmul(
                    v2Tps, lhsT=sb_xt[0:DIN, :], rhs=sb_w1b2,
                    start=True, stop=True,
                )
                v2T_sb = vup.tile([128, 128], DT.bfloat16, tag=f"v2T{g}")
                nc.vector.tensor_copy(out=v2T_sb, in_=v2Tps)
                uT_sbs.append(uT_sb)
                v2T_sbs.append(v2T_sb)

            # per-group accumulators
            raccs, rbs, rb_init = [], [], [False, False]
            for g in range(GROUPS_PER_CORE):
                racc = vup.tile([128, 2, N_DIR_PG], DT.float32, tag=f"racc{g}")
                raccs.append(racc)
                rb = vup.tile([128, 1024], DT.float16, tag=f"rb{g}")
                rbs.append(rb)
            dcount = [0, 0]

            l2ps = mega[:, 1024:2048]

            # ---- main lockstep pipeline ----
            for i in range(NITER):
                g = i % 2
                it = i // 2
                pre = mega[:, 512 * (i % 2) : 512 * (i % 2) + 512]
                # H1 on the PE: sel-matmul (K=64) + iden-matmul (K=128)
                nc.tensor.matmul(
                    pre, lhsT=uT_sbs[g], rhs=sb_sel[:, 512 * it : 512 * (it + 1)],
                    start=True, stop=False,
                )
                nc.tensor.matmul(
                    pre, lhsT=v2T_sbs[g], rhs=sb_iden, start=False, stop=True,
                )
                h1 = h1pool.tile([128, 512], DT.bfloat16)
                nc.scalar.activation(out=h1, in_=pre, func=RELU)

                # L2: 4 matmuls N=256 into the single l2 buffer
                nc.tensor.matmul(
                    l2ps[:, 0:256], lhsT=sb_w2a, rhs=h1[:, 0:256],
                    start=True, stop=True,
                )
                nc.tensor.matmul(
                    l2ps[:, 512:768], lhsT=sb_w2a, rhs=h1[:, 256:512],
                    start=True, stop=True,
                )
                nc.tensor.matmul(
                    l2ps[:, 256:512], lhsT=sb_w2b, rhs=h1[:, 0:256],
                    start=True, stop=True,
                )
                nc.tensor.matmul(
                    l2ps[:, 768:1024], lhsT=sb_w2b, rhs=h1[:, 256:512],
                    start=True, stop=True,
                )
                h2 = h2pool.tile([128, 1024], DT.bfloat16)
                nc.scalar.activation(
                    out=h2, in_=l2ps, func=RELU, bias=sb_b2c, scale=1.0
                )

                # L3: 4 matmuls N=512 into the two G slots (w3a then w3b
                # ordering halves the LDWEIGHTS count)
                g0 = mega[:, 2048:3072]
                g1 = mega[:, 3072:4096]
                nc.tensor.matmul(
                    g0[:, 0:512], lhsT=sb_w3a, rhs=h2[:, 0:512],
                    start=True, stop=True,
                )
                nc.tensor.matmul(
                    g1[:, 0:512], lhsT=sb_w3a, rhs=h2[:, 512:1024],
                    start=True, stop=True,
                )
                nc.tensor.matmul(
                    g0[:, 512:1024], lhsT=sb_w3b, rhs=h2[:, 0:512],
                    start=True, stop=True,
                )
                nc.tensor.matmul(
                    g1[:, 512:1024], lhsT=sb_w3b, rhs=h2[:, 512:1024],
                    start=True, stop=True,
                )

                # drains (per-slot [1024] ops)
                for sl, gp in enumerate((g0, g1)):
                    k = 2 * it + sl
                    if k % COPY_MOD == COPY_MOD - 1:
                        gc = gcpool.tile([128, 1024], DT.float16)
                        nc.scalar.copy(out=gc, in_=gp)
                        if not rb_init[g]:
                            rb_init[g] = True
                            nc.vector.tensor_copy(out=rbs[g], in_=gc)
                        else:
                            nc.vector.tensor_tensor(
                                out=rbs[g], in0=gc, in1=rbs[g], op=ALU.max
                            )
                    else:
                        t = dcount[g]
                        dcount[g] += 1
                        nc.vector.reduce_max(
                            out=raccs[g][:, :, t : t + 1],
                            in_=gp.rearrange("p (a b) -> p a b", a=2),
                            axis=AX.X,
                        )

            # ---- P per group, batched F-MLP (N=2); pb is (half, group) ----
            pb = fmlp.tile([128, 2, 2], DT.float16, tag="pb")
            for g in range(GROUPS_PER_CORE):
                pmA = fmlp.tile([128, 2], DT.float32, tag=f"pmA{g}")
                nc.vector.reduce_max(out=pmA, in_=raccs[g], axis=AX.X)
                pmB = fmlp.tile([128, 2], DT.float32, tag=f"pmB{g}")
                nc.vector.reduce_max(
                    out=pmB,
                    in_=rbs[g].rearrange("p (a b) -> p a b", a=2),
                    axis=AX.X,
                )
                pmx = fmlp.tile([128, 2], DT.float32, tag=f"pmx{g}")
                nc.vector.tensor_tensor(out=pmx, in0=pmA, in1=pmB, op=ALU.max)
                nc.vector.tensor_tensor(
                    out=pb[:, :, g], in0=pmx, in1=sb_b3_2, op=ALU.add
                )

            y1ps = mega[:, 0:8].rearrange("p (m g) -> p m g", m=4)
            for mm in range(4):
                for kk in range(2):
                    nc.tensor.matmul(
                        y1ps[:, mm, :],
                        lhsT=v1t(kk)[:, mm * 128 : (mm + 1) * 128],
                        rhs=pb[:, kk, :],
                        start=(kk == 0),
                        stop=(kk == 1),
                    )
            y1pre = fmlp.tile([128, 8], DT.float32, tag="y1pre")
            nc.vector.tensor_tensor(
                out=y1pre, in0=mega[:, 0:8], in1=sb_c1_42, op=ALU.add
            )
            y1 = fmlp.tile([128, 4, 2], DT.float16, tag="y1")
            nc.vector.tensor_scalar_max(
                out=y1.rearrange("p m g -> p (m g)"), in0=y1pre, scalar1=0.0
            )

            y2ps = mega[:, 1024:1028].rearrange("p (m g) -> p m g", m=2)
            for mm in range(2):
                for kk in range(4):
                    nc.tensor.matmul(
                        y2ps[:, mm, :],
                        lhsT=v2t(kk)[:, mm * 128 : (mm + 1) * 128],
                        rhs=y1[:, kk, :],
                        start=(kk == 0),
                        stop=(kk == 3),
                    )
            y2pre = fmlp.tile([128, 4], DT.float32, tag="y2pre")
            nc.vector.tensor_tensor(
                out=y2pre, in0=mega[:, 1024:1028], in1=sb_c2_22, op=ALU.add
            )
            y2 = fmlp.tile([128, 2, 2], DT.float16, tag="y2")
            nc.vector.tensor_scalar_max(
                out=y2.rearrange("p m g -> p (m g)"), in0=y2pre, scalar1=0.0
            )

            y3ps = mega[0:40, 2048:2050]
            for kk in range(2):
                nc.tensor.matmul(
                    y3ps,
                    lhsT=v3t(kk)[:, 0:40],
                    rhs=y2[:, kk, :],
                    start=(kk == 0),
                    stop=(kk == 1),
                )
            y3 = fmlp.tile([40, 2], DT.float32, tag="y3")
            nc.vector.tensor_scalar_add(out=y3, in0=y3ps, scalar1=sb_c3c)
            for g in range(GROUPS_PER_CORE):
                nc.sync.dma_start(out=y_out[g, :], in_=y3[:, g])

    _split_multi_waits(nc)
    return nc


# ---------------------------------------------------------------------------
# Host side
# ---------------------------------------------------------------------------
_NC_CACHE = None


def _get_program():
    global _NC_CACHE
    if _NC_CACHE is None:
        _NC_CACHE = _build_program()
    return _NC_CACHE


def _make_in_maps(inputs):
    X = np.asarray(inputs["X"], F32)
    W1 = np.asarray(inputs["W1"], F32)
    b1 = np.asarray(inputs["b1"], F32)
    W2 = np.asarray(inputs["W2"], F32)
    b2 = np.asarray(inputs["b2"], F32)
    W3 = np.asarray(inputs["W3"], F32)
    b3 = np.asarray(inputs["b3"], F32)
    V1 = np.asarray(inputs["V1"], F32)
    c1 = np.asarray(inputs["c1"], F32)
    V2 = np.asarray(inputs["V2"], F32)
    c2 = np.asarray(inputs["c2"], F32)
    V3 = np.asarray(inputs["V3"], F32)
    c3 = np.asarray(inputs["c3"], F32)

    W1A, W1B = W1[:, :DIN], W1[:, DIN:]
    # u-rhs: [DIN+1, 64] = [W1A^T; b1]; w1b2: [DIN, 128] doubled (pad row 0)
    w1blob = np.zeros((DIN + 1, 192), F32)
    w1blob[0:DIN, 0:64] = W1A.T
    w1blob[DIN, 0:64] = b1
    w1blob[0:DIN, 64:192] = np.concatenate([W1B.T, W1B.T], axis=1)
    w1blob = w1blob.astype(BF16)

    # sel[jp, it*512 + q*128 + i] = (jp == 4*it + q)
    selblob = np.zeros((JP, (NITER // 2) * 512), F32)
    for jp in range(JP):
        it, q = jp // 4, jp % 4
        selblob[jp, it * 512 + q * 128 : it * 512 + (q + 1) * 128] = 1.0
    selblob = selblob.astype(BF16)
    # iden[i', q*128 + i] = (i' == i)
    idenblob = np.tile(np.eye(M, dtype=F32), (1, 4)).astype(BF16)

    z64 = np.zeros((64, 128), F32)
    wblob = np.concatenate(
        [
            np.concatenate([W2.T, z64], axis=0),
            np.concatenate([z64, W2.T], axis=0),
            W3.T[:, 0:128],
            W3.T[:, 128:256],
        ],
        axis=1,
    ).astype(BF16)
    v1t_cols = V1.T.reshape(2, 128, 512).transpose(1, 0, 2).reshape(128, 1024)
    vblob = np.concatenate(
        [v1t_cols,
         V2.T.reshape(4, 128, 256).transpose(1, 0, 2).reshape(128, 1024),
         V3.T.reshape(2, 128, 40).transpose(1, 0, 2).reshape(128, 80)],
        axis=1,
    ).astype(np.float16)
    cblob = np.zeros((128, 17), F32)
    cblob[:, 1] = b2
    cblob[:, 2:4] = b3.reshape(2, 128).T
    cblob[:, 4:12] = np.repeat(c1.reshape(4, 128).T, 2, axis=1)
    cblob[:, 12:16] = np.repeat(c2.reshape(2, 128).T, 2, axis=1)
    cblob[0:40, 16] = c3

    shared = dict(
        wblob=wblob, w1blob=w1blob, vblob=vblob, cblob=cblob,
        selb=selblob, idenb=idenblob,
    )

    Xv = X.reshape(B, D, M, DIN)
    in_maps = []
    for c in range(N_CORES):
        xts = np.ones((GROUPS_PER_CORE, DIN + 1, M), F32)
        for gi in range(GROUPS_PER_CORE):
            g = 2 * c + gi
            bb, dd = g // D, g % D
            xts[gi, 0:DIN] = Xv[bb, dd].T
        in_maps.append(dict(shared, xt=xts.astype(BF16)))
    return in_maps


def _run(inputs, trace=False):
    nc = _get_program()
    in_maps = _make_in_maps(inputs)
    res = run_bass_kernel_spmd(nc, in_maps, list(range(N_CORES)), trace=trace)
    ys = np.stack([res.results[c]["y"] for c in range(N_CORES)])  # [8, 2, 40]
    y16 = ys.reshape(B, D, 40)
    out = y16.max(axis=1).astype(F32)
    return out, res


def kernel(**inputs):
    out, _ = _run(inputs, trace=False)
    return out


# revision 12
# speedup vs baseline: 1.4107x; 1.2471x over previous
"""Trainium2 Bass kernel for BoostedPointPairNet2.

Model (per (b, d) group, m = 128 points, din = 3):
  H1(i,j) = relu(W1A @ x_j + W1B @ x_i + b1)          (64)
  H2(i,j) = relu(W2 @ H1 + b2)                        (128)
  G(i,j)  = W3 @ H2                                    (256, b3 deferred)
  P       = max_{i,j} G + b3                           (256)
  Y       = V3 @ relu(V2 @ relu(V1 @ P + c1) + c2) + c3  (40)
  out[b]  = max_d Y[b, d]

Sharding: 16 (b, d) groups over 8 cores, 2 groups per core; host does the
final max over d.

v3 dataflow — H1 computed ON THE PE via selection-matrix matmuls:
  pre_H1[(e,ch), (q,i)] = sum_jp uT[jp, (e,ch)] * sel[jp, (q,i)]   (K=64)
                        + sum_i' v2T[i', (e,ch)] * iden[i', (q,i)] (K=128)
  where sel is the 0/1 matrix delta(jp == 4*it+q) and iden is I_128
  tiled 4x along columns; b1 is folded into uT via an extra ones row in
  the x input.  One [512]-col relu (ACT) then yields the bf16 stacked h1
  tile, replacing 128 per-j-pair elementwise ops (~48us of engine time)
  with ~14us of otherwise-idle PE time.

  The two groups run lockstep (even iter -> group 0, odd -> group 1),
  4 j-pairs per iteration.  PSUM [128, 4096] fp32 mega-tile layout:
    cols    0:1024  two [512] pre_H1 buffers (banks 0-1)
    cols 1024:2048  single l2 buffer (banks 2-3)
    cols 2048:4096  two [1024] G slots (banks 4-7)
  G drains are per-slot [1024] ops so the DVE stays throughput-bound:
  most tiles are direct DVE reduce_max into per-group racc columns; every
  4th per-group tile takes the ACT-copy (fp16) + DVE tensor_tensor-max
  path to balance the engines (~75us each).  The F-MLP tail is batched
  across both groups (N=2 matmuls) and runs once at the end.
"""

import numpy as np
import ml_dtypes

import bass_rust
import concourse.bass as bass
import concourse.mybir as mybir
from concourse.tile import TileContext
from concourse.bass_utils import run_bass_kernel_spmd

BF16 = ml_dtypes.bfloat16
F32 = np.float32
DT = mybir.dt
ALU = mybir.AluOpType
AX = mybir.AxisListType
RELU = mybir.ActivationFunctionType.Relu

N_CORES = 8
B, N, DIN = 4, 512, 3
D = 4                    # boost factor
M = N // D               # 128 points per group
GROUPS_PER_CORE = 2
JP = M // 2              # 64 stacked j-pairs per group
NITER = 32               # lockstep iterations (16 per group, 4 jp each)
NWARM = 26               # warmup junk matmuls to lift the HAM clock gate
COPY_MOD = 4             # per-group drain k: ACT-copy path if k % COPY_MOD == 3
N_COPY_PG = sum(1 for k in range(NITER) if k % COPY_MOD == 3)
N_DIR_PG = NITER - N_COPY_PG


def _split_multi_waits(nc):
    """This walrus build accepts at most ONE sync wait per instruction;
    hoist extra waits onto same-engine nops inserted before the offender."""
    seq = 0
    for fn in nc.m.functions:
        for bb in fn.blocks:
            new = []
            changed = False
            for ins in bb.instructions:
                si = ins.sync_info
                waits = list(si.on_wait) if si is not None and si.on_wait else []
                if len(waits) > 1:
                    changed = True
                    for w in waits[:-1]:
                        seq += 1
                        new.append(
                            mybir.InstNoOp(
                                name=f"I-wsplit-{seq}",
                                engine=ins.engine,
                                sync_info=bass_rust.SyncInfo(
                                    on_wait=[w], on_update=[]
                                ),
                            )
                        )
                    ins.sync_info = bass_rust.SyncInfo(
                        on_wait=[waits[-1]], on_update=list(si.on_update or [])
                    )
                new.append(ins)
            if changed:
                bb.instructions = new


# ---------------------------------------------------------------------------
# Device program
# ---------------------------------------------------------------------------
def _build_program():
    nc = bass.Bass(
        "TRN2", target_bir_lowering=False, debug=False, num_devices=N_CORES
    )

    # x with an appended ones row (folds b1 into the uT prep matmul)
    xt = nc.declare_dram_parameter(
        "xt", [GROUPS_PER_CORE, DIN + 1, M], DT.bfloat16, isOutput=False
    )
    # col 0:128 w2a=[W2T;0], 128:256 w2b=[0;W2T], 256:384 w3a, 384:512 w3b
    wblob = nc.declare_dram_parameter("wblob", [128, 512], DT.bfloat16, isOutput=False)
    # cols 0:64 u-rhs ([W1A^T; b1]), cols 64:192 w1b2 ([W1B^T|W1B^T]; 0)
    w1blob = nc.declare_dram_parameter("w1blob", [DIN + 1, 192], DT.bfloat16, isOutput=False)
    # sel[jp, it*512 + q*128 + i] = (jp == 4*it + q)
    selb = nc.declare_dram_parameter("selb", [JP, (NITER // 2) * 512], DT.bfloat16, isOutput=False)
    # iden[i', q*128 + i] = (i' == i)
    idenb = nc.declare_dram_parameter("idenb", [M, 4 * M], DT.bfloat16, isOutput=False)
    # v1t (2x512) | v2t (4x256) | v3t (2x40)
    vblob = nc.declare_dram_parameter("vblob", [128, 2128], DT.float16, isOutput=False)
    # col 1 b2c, 2:4 b3_2, 4:12 c1_42 (4mm x 2g), 12:16 c2_22, 16 c3
    cblob = nc.declare_dram_parameter("cblob", [128, 17], DT.float32, isOutput=False)
    y_out = nc.declare_dram_parameter(
        "y", [40, GROUPS_PER_CORE], DT.float32, isOutput=True
    )

    with TileContext(nc) as tc:
        with (
            tc.tile_pool(name="singles", bufs=1) as singles,
            tc.tile_pool(name="xtp", bufs=2) as xtp,
            tc.tile_pool(name="vup", bufs=8) as vup,
            tc.tile_pool(name="h1p", bufs=3) as h1pool,
            tc.tile_pool(name="h2p", bufs=3) as h2pool,
            tc.tile_pool(name="gcp", bufs=3) as gcpool,
            tc.tile_pool(name="fmlp", bufs=12) as fmlp,
            tc.tile_pool(name="psum", bufs=1, space="PSUM") as psum,
        ):
            # The whole PSUM as one tile.
            mega = psum.tile([128, 4096], DT.float32, tag="mega")

            # ---- input DMAs (sync + gpsimd queues; ACT/DVE stay free).
            # sel is 1MB: split into 4 chunks so iter 0 isn't gated on it ----
            sb_sel = singles.tile([JP, (NITER // 2) * 512], DT.bfloat16, tag="selb")
            nc.sync.dma_start(out=sb_sel[:, 0:2048], in_=selb[:, 0:2048])
            sb_xts = []
            for g in range(GROUPS_PER_CORE):
                t = xtp.tile([DIN + 1, M], DT.bfloat16)
                sb_xts.append(t)
            nc.gpsimd.dma_start(out=sb_xts[0], in_=xt[0])
            sb_w1 = singles.tile([DIN + 1, 192], DT.bfloat16, tag="w1blob")
            nc.gpsimd.dma_start(out=sb_w1, in_=w1blob[:, :])
            sb_iden = singles.tile([M, 4 * M], DT.bfloat16, tag="idenb")
            nc.gpsimd.dma_start(out=sb_iden, in_=idenb[:, :])
            sb_c = singles.tile([128, 17], DT.float32, tag="cblob")
            nc.gpsimd.dma_start(out=sb_c, in_=cblob[:, :])
            sb_w = singles.tile([128, 512], DT.bfloat16, tag="wblob")
            nc.gpsimd.dma_start(out=sb_w, in_=wblob[:, :])
            nc.gpsimd.dma_start(out=sb_xts[1], in_=xt[1])
            nc.sync.dma_start(out=sb_sel[:, 2048:4096], in_=selb[:, 2048:4096])
            nc.sync.dma_start(out=sb_sel[:, 4096:6144], in_=selb[:, 4096:6144])
            nc.sync.dma_start(out=sb_sel[:, 6144:8192], in_=selb[:, 6144:8192])
            sb_v = singles.tile([128, 2128], DT.float16, tag="vblob")
            nc.gpsimd.dma_start(out=sb_v, in_=vblob[:, :])

            # ---- PE warmup burst (junk matmuls while DMAs land) ----
            wjunk = singles.tile([128, 128], DT.bfloat16, tag="wjunk")
            nc.gpsimd.memset(wjunk, 0.0)
            for _ in range(NWARM):
                nc.tensor.matmul(
                    mega[:, 3200:3328], lhsT=wjunk, rhs=wjunk,
                    start=True, stop=True,
                )

            # dummy relu to hoist ACT_TABLE_LOAD into the init shadow
            warm = singles.tile([1, 1], DT.float32, tag="warm")
            nc.gpsimd.memset(warm, 0.0)
            nc.scalar.activation(out=warm, in_=warm, func=RELU)

            sb_w2a, sb_w2b = sb_w[:, 0:128], sb_w[:, 128:256]
            sb_w3a, sb_w3b = sb_w[:, 256:384], sb_w[:, 384:512]
            sb_urhs = sb_w1[:, 0:64]
            sb_w1b2 = sb_w1[0:DIN, 64:192]
            sb_b2c = sb_c[:, 1:2]
            sb_b3_2 = sb_c[:, 2:4]
            sb_c1_42 = sb_c[:, 4:12]
            sb_c2_22 = sb_c[:, 12:16]
            sb_c3c = sb_c[0:40, 16:17]

            def v1t(k):  # [128, 512] fp16, k in 0..1
                return sb_v[:, 512 * k : 512 * (k + 1)]

            def v2t(k):  # [128, 256] fp16, k in 0..3
                return sb_v[:, 1024 + 256 * k : 1024 + 256 * (k + 1)]

            def v3t(k):  # [128, 40] fp16, k in 0..1
                return sb_v[:, 2048 + 40 * k : 2048 + 40 * (k + 1)]

            # ---- per-group prep: uT [64,128] and v2T [128,128] in SBUF bf16
            # (psum carved from the G regions, consumed before first L3) ----
            uT_sbs, v2T_sbs = [], []
            for g in range(GROUPS_PER_CORE):
                sb_xt = sb_xts[g]
                xt_eo = sb_xt.rearrange("k (j two) -> k two j", two=2)
                uTps = mega[0:JP, 2048 + 512 * g : 2048 + 512 * g + 128]
                nc.tensor.matmul(
                    uTps[:, 0:64], lhsT=xt_eo[:, 0, :], rhs=sb_urhs,
                    start=True, stop=True,
                )
                nc.tensor.matmul(
                    uTps[:, 64:128], lhsT=xt_eo[:, 1, :], rhs=sb_urhs,
                    start=True, stop=True,
                )
                uT_sb = vup.tile([JP, 128], DT.bfloat16, tag=f"uT{g}")
                nc.vector.tensor_copy(out=uT_sb, in_=uTps)
                v2Tps = mega[:, 3072 + 512 * g : 3072 + 512 * g + 128]
                nc.tensor.matmul(
                    v2Tps, lhsT=sb_xt[0:DIN, :], rhs=sb_w1b2,
                    start=True, stop=True,
                )
                v2T_sb = vup.tile([128, 128], DT.bfloat16, tag=f"v2T{g}")
                nc.vector.tensor_copy(out=v2T_sb, in_=v2Tps)
                uT_sbs.append(uT_sb)
                v2T_sbs.append(v2T_sb)

            # per-group accumulators
            raccs, rbs, rb_init = [], [], [False, False]
            for g in range(GROUPS_PER_CORE):
                racc = vup.tile([128, 2, N_DIR_PG], DT.float32, tag=f"racc{g}")
                raccs.append(racc)
                rb = vup.tile([128, 1024], DT.float16, tag=f"rb{g}")
                rbs.append(rb)
            dcount = [0, 0]

            l2ps = mega[:, 1024:2048]

            def issue_sel_h1(i):
                g, it = i % 2, i // 2
                pre = mega[:, 512 * (i % 2) : 512 * (i % 2) + 512]
                nc.tensor.matmul(
                    pre, lhsT=uT_sbs[g],
                    rhs=sb_sel[:, 512 * it : 512 * (it + 1)],
                    start=True, stop=False,
                )
                nc.tensor.matmul(
                    pre, lhsT=v2T_sbs[g], rhs=sb_iden, start=False, stop=True,
                )
                h1 = h1pool.tile([128, 512], DT.bfloat16)
                nc.scalar.activation(out=h1, in_=pre, func=RELU)
                return h1

            def issue_l2(h1):
                nc.tensor.matmul(
                    l2ps[:, 0:256], lhsT=sb_w2a, rhs=h1[:, 0:256],
                    start=True, stop=True,
                )
                nc.tensor.matmul(
                    l2ps[:, 512:768], lhsT=sb_w2a, rhs=h1[:, 256:512],
                    start=True, stop=True,
                )
                nc.tensor.matmul(
                    l2ps[:, 256:512], lhsT=sb_w2b, rhs=h1[:, 0:256],
                    start=True, stop=True,
                )
                nc.tensor.matmul(
                    l2ps[:, 768:1024], lhsT=sb_w2b, rhs=h1[:, 256:512],
                    start=True, stop=True,
                )

            # ---- main lockstep pipeline (1-iteration software skew so the
            # PE queue runs L2(i+1) ahead of L3(i): the l2->relu->l2 recycle
            # is the latency-critical cycle, the G drains are not) ----
            h1s = {}
            h1s[0] = issue_sel_h1(0)
            issue_l2(h1s[0])
            h1s[1] = issue_sel_h1(1)
            for i in range(NITER):
                g, it = i % 2, i // 2
                h2 = h2pool.tile([128, 1024], DT.bfloat16)
                nc.scalar.activation(
                    out=h2, in_=l2ps, func=RELU, bias=sb_b2c, scale=1.0
                )
                if i + 1 < NITER:
                    issue_l2(h1s[i + 1])
                if i + 2 < NITER:
                    h1s[i + 2] = issue_sel_h1(i + 2)

                # L3: 4 matmuls N=512 into the two G slots (w3a then w3b
                # ordering halves the LDWEIGHTS count)
                g0 = mega[:, 2048:3072]
                g1 = mega[:, 3072:4096]
                nc.tensor.matmul(
                    g0[:, 0:512], lhsT=sb_w3a, rhs=h2[:, 0:512],
                    start=True, stop=True,
                )
                nc.tensor.matmul(
                    g1[:, 0:512], lhsT=sb_w3a, rhs=h2[:, 512:1024],
                    start=True, stop=True,
                )
                nc.tensor.matmul(
                    g0[:, 512:1024], lhsT=sb_w3b, rhs=h2[:, 0:512],
                    start=True, stop=True,
                )
                nc.tensor.matmul(
                    g1[:, 512:1024], lhsT=sb_w3b, rhs=h2[:, 512:1024],
                    start=True, stop=True,
                )

                # drains (per-slot [1024] ops)
                for sl, gp in enumerate((g0, g1)):
                    k = 2 * it + sl
                    if k % COPY_MOD == COPY_MOD - 1:
                        gc = gcpool.tile([128, 1024], DT.float16)
                        nc.scalar.copy(out=gc, in_=gp)
                        if not rb_init[g]:
                            rb_init[g] = True
                            nc.vector.tensor_copy(out=rbs[g], in_=gc)
                        else:
                            nc.vector.tensor_tensor(
                                out=rbs[g], in0=gc, in1=rbs[g], op=ALU.max
                            )
                    else:
                        t = dcount[g]
                        dcount[g] += 1
                        nc.vector.reduce_max(
                            out=raccs[g][:, :, t : t + 1],
                            in_=gp.rearrange("p (a b) -> p a b", a=2),
                            axis=AX.X,
                        )

            # ---- P per group, batched F-MLP (N=2); pb is (half, group) ----
            pb = fmlp.tile([128, 2, 2], DT.float16, tag="pb")
            for g in range(GROUPS_PER_CORE):
                pmA = fmlp.tile([128, 2], DT.float32, tag=f"pmA{g}")
                nc.vector.reduce_max(out=pmA, in_=raccs[g], axis=AX.X)
                pmB = fmlp.tile([128, 2], DT.float32, tag=f"pmB{g}")
                nc.vector.reduce_max(
                    out=pmB,
                    in_=rbs[g].rearrange("p (a b) -> p a b", a=2),
                    axis=AX.X,
                )
                pmx = fmlp.tile([128, 2], DT.float32, tag=f"pmx{g}")
                nc.vector.tensor_tensor(out=pmx, in0=pmA, in1=pmB, op=ALU.max)
                nc.vector.tensor_tensor(
                    out=pb[:, :, g], in0=pmx, in1=sb_b3_2, op=ALU.add
                )

            y1ps = mega[:, 0:8].rearrange("p (m g) -> p m g", m=4)
            for mm in range(4):
                for kk in range(2):
                    nc.tensor.matmul(
                        y1ps[:, mm, :],
                        lhsT=v1t(kk)[:, mm * 128 : (mm + 1) * 128],
                        rhs=pb[:, kk, :],
                        start=(kk == 0),
                        stop=(kk == 1),
                    )
            y1pre = fmlp.tile([128, 8], DT.float32, tag="y1pre")
            nc.vector.tensor_tensor(
                out=y1pre, in0=mega[:, 0:8], in1=sb_c1_42, op=ALU.add
            )
            y1 = fmlp.tile([128, 4, 2], DT.float16, tag="y1")
            nc.vector.tensor_scalar_max(
                out=y1.rearrange("p m g -> p (m g)"), in0=y1pre, scalar1=0.0
            )

            y2ps = mega[:, 1024:1028].rearrange("p (m g) -> p m g", m=2)
            for mm in range(2):
                for kk in range(4):
                    nc.tensor.matmul(
                        y2ps[:, mm, :],
                        lhsT=v2t(kk)[:, mm * 128 : (mm + 1) * 128],
                        rhs=y1[:, kk, :],
                        start=(kk == 0),
                        stop=(kk == 3),
                    )
            y2pre = fmlp.tile([128, 4], DT.float32, tag="y2pre")
            nc.vector.tensor_tensor(
                out=y2pre, in0=mega[:, 1024:1028], in1=sb_c2_22, op=ALU.add
            )
            y2 = fmlp.tile([128, 2, 2], DT.float16, tag="y2")
            nc.vector.tensor_scalar_max(
                out=y2.rearrange("p m g -> p (m g)"), in0=y2pre, scalar1=0.0
            )

            y3ps = mega[0:40, 2048:2050]
            for kk in range(2):
                nc.tensor.matmul(
                    y3ps,
                    lhsT=v3t(kk)[:, 0:40],
                    rhs=y2[:, kk, :],
                    start=(kk == 0),
                    stop=(kk == 1),
                )
            y3 = fmlp.tile([40, 2], DT.float32, tag="y3")
            nc.vector.tensor_scalar_add(out=y3, in0=y3ps, scalar1=sb_c3c)
            nc.sync.dma_start(out=y_out[:, :], in_=y3)

    _split_multi_waits(nc)
    return nc


# ---------------------------------------------------------------------------
# Host side
# ---------------------------------------------------------------------------
_NC_CACHE = None


def _get_program():
    global _NC_CACHE
    if _NC_CACHE is None:
        _NC_CACHE = _build_program()
    return _NC_CACHE


def _make_in_maps(inputs):
    X = np.asarray(inputs["X"], F32)
    W1 = np.asarray(inputs["W1"], F32)
    b1 = np.asarray(inputs["b1"], F32)
    W2 = np.asarray(inputs["W2"], F32)
    b2 = np.asarray(inputs["b2"], F32)
    W3 = np.asarray(inputs["W3"], F32)
    b3 = np.asarray(inputs["b3"], F32)
    V1 = np.asarray(inputs["V1"], F32)
    c1 = np.asarray(inputs["c1"], F32)
    V2 = np.asarray(inputs["V2"], F32)
    c2 = np.asarray(inputs["c2"], F32)
    V3 = np.asarray(inputs["V3"], F32)
    c3 = np.asarray(inputs["c3"], F32)

    W1A, W1B = W1[:, :DIN], W1[:, DIN:]
    # u-rhs: [DIN+1, 64] = [W1A^T; b1]; w1b2: [DIN, 128] doubled (pad row 0)
    w1blob = np.zeros((DIN + 1, 192), F32)
    w1blob[0:DIN, 0:64] = W1A.T
    w1blob[DIN, 0:64] = b1
    w1blob[0:DIN, 64:192] = np.concatenate([W1B.T, W1B.T], axis=1)
    w1blob = w1blob.astype(BF16)

    # sel[jp, it*512 + q*128 + i] = (jp == 4*it + q)
    selblob = np.zeros((JP, (NITER // 2) * 512), F32)
    for jp in range(JP):
        it, q = jp // 4, jp % 4
        selblob[jp, it * 512 + q * 128 : it * 512 + (q + 1) * 128] = 1.0
    selblob = selblob.astype(BF16)
    # iden[i', q*128 + i] = (i' == i)
    idenblob = np.tile(np.eye(M, dtype=F32), (1, 4)).astype(BF16)

    z64 = np.zeros((64, 128), F32)
    wblob = np.concatenate(
        [
            np.concatenate([W2.T, z64], axis=0),
            np.concatenate([z64, W2.T], axis=0),
            W3.T[:, 0:128],
            W3.T[:, 128:256],
        ],
        axis=1,
    ).astype(BF16)
    v1t_cols = V1.T.reshape(2, 128, 512).transpose(1, 0, 2).reshape(128, 1024)
    vblob = np.concatenate(
        [v1t_cols,
         V2.T.reshape(4, 128, 256).transpose(1, 0, 2).reshape(128, 1024),
         V3.T.reshape(2, 128, 40).transpose(1, 0, 2).reshape(128, 80)],
        axis=1,
    ).astype(np.float16)
    cblob = np.zeros((128, 17), F32)
    cblob[:, 1] = b2
    cblob[:, 2:4] = b3.reshape(2, 128).T
    cblob[:, 4:12] = np.repeat(c1.reshape(4, 128).T, 2, axis=1)
    cblob[:, 12:16] = np.repeat(c2.reshape(2, 128).T, 2, axis=1)
    cblob[0:40, 16] = c3

    shared = dict(
        wblob=wblob, w1blob=w1blob, vblob=vblob, cblob=cblob,
        selb=selblob, idenb=idenblob,
    )

    Xv = X.reshape(B, D, M, DIN)
    in_maps = []
    for c in range(N_CORES):
        xts = np.ones((GROUPS_PER_CORE, DIN + 1, M), F32)
        for gi in range(GROUPS_PER_CORE):
            g = 2 * c + gi
            bb, dd = g // D, g % D
            xts[gi, 0:DIN] = Xv[bb, dd].T
        in_maps.append(dict(shared, xt=xts.astype(BF16)))
    return in_maps


def _run(inputs, trace=False):
    nc = _get_program()
    in_maps = _make_in_maps(inputs)
    res = run_bass_kernel_spmd(nc, in_maps, list(range(N_CORES)), trace=trace)
    ys = np.stack([res.results[c]["y"].T for c in range(N_CORES)])  # [8, 2, 40]
    y16 = ys.reshape(B, D, 40)
    out = y16.max(axis=1).astype(F32)
    return out, res


def kernel(**inputs):
    out, _ = _run(inputs, trace=False)
    return out
